# revision 1
# baseline (speedup 1.0000x reference)
"""Trainium2 Bass kernel for CantorMultiheadFusion.

Reference math:
    h      = x @ W_in^T                        # [B,S,D]
    d[s,k] = distances[s, routes[s,k]]
    w      = softmax(-d, axis=-1)              # [S,K]
    fused  = sum_k w[s,k] * h[:, routes[s,k]]  # [B,S,D]  (head reshape is a no-op)
    out    = fused @ W_out^T + b_out + x

Because the fusion weights are shared across the feature dim, the gather
commutes with both projections:
    out = (A @ x) @ (W_out @ W_in)^T + b_out + x
where A[s,j] = C[s,j] * exp(-distances[s,j]) / denom(s),
      C[s,j] = #{k : routes[s,k] == j}   (integer multiplicity),
      denom(s) = sum_j C[s,j] * exp(-distances[s,j]).
Duplicated route entries share the same distance, so the count matrix C is
exact. On device this is computed as exp(ln(C) - D) with ln(C) built
host-side from the int32 routes tensor alone (index marshalling; C=0 maps
to -1e4 so exp underflows to +0). All float math on the actual inputs
(exp, normalization, matmuls, residual) runs on device.

Sharding: sequence-parallel over S across 8 cores (256 rows each). x is
replicated since the A@x contraction needs all S rows. Layouts keep the
contraction dim on partitions with no on-chip transposes:
  stepA: t^T[e,s] += x[b,j-tile](lhsT) @ numerT[j-tile](rhs)   (j contract)
  stepB: out[s,i] += tT[s-chunk](lhsT) @ WcT(rhs)              (e contract)
stepA packs two accumulation groups per PSUM bank using a single
start=True per bank (start clears the whole bank's has_written bits, so
the second group relies on cleared bits to overwrite on first write) —
this lets both batches accumulate concurrently in 4 banks.

Precision plan (host casts are pure dtype marshalling): the matmul
datapath (x, softmax numerator, distances, ln(C), weights) streams in
bf16; the residual path and output stay fp32. Measured rel err ~5e-4.
Set STEPA_FP8=True to switch the x/numer stream to fp8e4m3 (~3us faster,
rel err ~5e-3).
"""

import os
import sys

import numpy as np

for _p in ("/opt/trn_rl_repo",):
    if os.path.isdir(_p) and _p not in sys.path:
        sys.path.insert(0, _p)

# Some container snapshots lack antenv.axon_hooks (the axon NTFF profile
# hook); stub it so run_bass_kernel_spmd(trace=True) degrades gracefully.
def _ensure_axon_hooks_stub():
    import types
    try:
        import antenv.axon_hooks  # noqa: F401
    except ModuleNotFoundError:
        try:
            import antenv
        except ModuleNotFoundError:
            return
        _stub = types.ModuleType("antenv.axon_hooks")
        _stub.get_axon_ntff_profile_hook = lambda: None
        sys.modules["antenv.axon_hooks"] = _stub
        antenv.axon_hooks = _stub


_ensure_axon_hooks_stub()

B, S, D, K = 2, 2048, 512, 64
N_CORES = 8
SLOC = S // N_CORES          # 256 sequence rows per core
NJ = S // 128                # 16 contraction tiles
NE = D // 128                # 4 feature chunks
NSC = SLOC // 128            # 2 seq chunks per core
JG = 4                       # j-tiles per streamed DMA group
STEPA_FP8 = False             # fp8e4m3 vs bf16 for the x/numer stream
NJG = NJ // JG               # stream groups

_CACHE = {}
LAST_RESULTS = None


def _build_nc(with_bias=True):
    import concourse.bacc as bacc
    import concourse.mybir as mybir
    import concourse.tile as tile

    F32 = mybir.dt.float32
    BF16 = mybir.dt.bfloat16
    F8 = mybir.dt.float8e4 if STEPA_FP8 else mybir.dt.bfloat16
    MUL = mybir.AluOpType.mult
    ADD = mybir.AluOpType.add

    nc = bacc.Bacc("TRN2", target_bir_lowering=False, debug=False, num_devices=1)

    x_d = nc.dram_tensor("x", [B, S, D], F8, kind="ExternalInput").ap()
    distT_d = nc.dram_tensor("distT", [S, SLOC], BF16, kind="ExternalInput").ap()
    lnct_d = nc.dram_tensor("lnct", [S, SLOC], BF16, kind="ExternalInput").ap()
    xres_d = nc.dram_tensor("xres", [B, SLOC, D], F32, kind="ExternalInput").ap()
    win_d = nc.dram_tensor("w_in", [D, D], BF16, kind="ExternalInput").ap()
    woutT_d = nc.dram_tensor("w_outT", [D, D], BF16, kind="ExternalInput").ap()
    bout_d = nc.dram_tensor("b_out", [1, D], F32, kind="ExternalInput").ap()
    out_d = nc.dram_tensor("out", [B, SLOC, D], F32, kind="ExternalOutput").ap()

    with tile.TileContext(nc) as tc:
        with (
            tc.tile_pool(name="big", bufs=1) as big,
            tc.tile_pool(name="dstream", bufs=2) as dstream,
            tc.tile_pool(name="cstream", bufs=2) as cstream,
            tc.tile_pool(name="sstream", bufs=2) as sstream,
            tc.tile_pool(name="pa", bufs=4, space="PSUM") as pa,
            tc.tile_pool(name="pden", bufs=1, space="PSUM") as pdenp,
            tc.tile_pool(name="ptr", bufs=2, space="PSUM") as ptr,
        ):
            # ---- persistent SBUF ----
            xbuf = big.tile([128, B * NJ * D], F8)        # (b,j) -> [128j, 512e]
            numer = big.tile([128, NJ * SLOC], F8)        # (j)   -> [128j, 256s]
            win_sb = big.tile([128, 4 * D], BF16)         # (a)   -> [128a, 512e]
            woutT_sb = big.tile([128, 4 * D], BF16)       # (a)   -> [128a, 512i]
            wcT = big.tile([128, 4 * D], BF16)            # (e)   -> [128e, 512i]
            bias_sb = big.tile([1, D], F32)
            bias_bc = big.tile([128, D], F32)
            ones_r = big.tile([1, 128], F32)
            onescol_8 = big.tile([128, 1], F8)
            xres_sb = big.tile([128, 2 * B * D], F32)     # (b,sc) -> [128s, 512e]
            resb = big.tile([128, 2 * B * D], F32)        # xres + bias
            tT = big.tile([128, B * NE * SLOC], BF16)     # (b,ec) -> [128e, 256s]
            outbuf = big.tile([128, 2 * B * D], F32)      # (b,sc) -> [128s, 512i]
            rdT = big.tile([128, NSC], F32)

            def xsl(b, j, n=1):
                o = (b * NJ + j) * D
                return xbuf[:, o:o + n * D]

            def nsl(j, n=1):
                return numer[:, j * SLOC:(j + n) * SLOC]

            def tsl(buf, b, c):
                o = (b * NE + c) * SLOC
                return buf[:, o:o + SLOC]

            def bsl(buf, b, sc):
                o = (b * NSC + sc) * D
                return buf[:, o:o + D]

            nc.vector.memset(onescol_8[:], 1.0)
            nc.vector.memset(ones_r[:], 1.0)

            # stepA PSUM: bank (b,p) holds ec=2p (cols :SLOC) and ec=2p+1
            # (cols SLOC:). Exactly one start=True per bank (j==0, even ec).
            pdT = pdenp.tile([128, NSC], F32)
            pts = {(b, p): pa.tile([128, 2 * SLOC], F32,
                                   name=f"pts{b}_{p}", tag="acc")
                   for b in range(B) for p in range(NE // 2)}

            def pta(b, ec):
                return pts[(b, ec // 2)][:, (ec % 2) * SLOC:(ec % 2 + 1) * SLOC]

            # ---- streamed inputs: ONE queue (SP) so the serial DMA device
            # serves in exactly this order: bias, d0,c0,x00,x10, W,W,
            # d1,c1,x01,x11, d2,... The weights ride in the first gap so
            # WcT fills PE idle time after stepA group 0. ----
            if with_bias:
                nc.sync.dma_start(out=bias_sb[:1, :], in_=bout_d[:, :])
                pb = ptr.tile([128, D], F32, name="pb", tag="tr")
                nc.tensor.matmul(pb[:], lhsT=ones_r[:1, :], rhs=bias_sb[:1, :],
                                 start=True, stop=True)
                nc.vector.tensor_copy(bias_bc[:], pb[:])
            else:
                # b_out is all-zero for this invocation (host-dispatched
                # program variant): the bias DMA/broadcast and residual
                # pre-adds are skipped; the epilogue adds xres directly.
                nc.vector.memset(bias_sb[:1, :], 0.0)

            # Variable-size stream groups: small first groups let stepA
            # start sooner; the final group de-interleaves the batches so
            # b=0's PSUM stop + tT copies + reciprocal hide under b=1's MMs.
            GROUPS = [(0, 1), (1, 2), (3, 3), (6, 3), (9, 2), (11, 2), (13, 3)]
            LASTG = len(GROUPS) - 1

            def stepa_j(j, b):
                if b == 0:
                    for sc in range(NSC):
                        nc.tensor.matmul(
                            pdT[:, sc:sc + 1],
                            lhsT=nsl(j)[:, sc * 128:(sc + 1) * 128],
                            rhs=onescol_8[:, :1],
                            start=(j == 0 and sc == 0),
                            stop=(j == NJ - 1 and sc == NSC - 1),
                            skip_group_check=True)
                for ec in range(NE):
                    nc.tensor.matmul(
                        pta(b, ec),
                        lhsT=xsl(b, j)[:, ec * 128:(ec + 1) * 128],
                        rhs=nsl(j),
                        start=(j == 0 and ec % 2 == 0),
                        stop=(j == NJ - 1 and ec % 2 == 1),
                        skip_group_check=True)

            def tt_copies(b, eng):
                for p in range(NE // 2):
                    dst = tT[:, (b * NE + 2 * p) * SLOC:
                             (b * NE + 2 * p + 2) * SLOC]
                    if eng == "dve":
                        nc.vector.tensor_copy(dst, pts[(b, p)][:])
                    else:
                        nc.scalar.copy(dst, pts[(b, p)][:])

            for g, (j0, jn) in enumerate(GROUPS):
                dt_t = dstream.tile([128, jn * SLOC], BF16, name=f"dt{g}", tag="dt")
                nc.sync.dma_start(
                    out=dt_t[:].rearrange("p (j s) -> p j s", j=jn),
                    in_=distT_d[j0 * 128:(j0 + jn) * 128, :]
                        .rearrange("(j p) s -> p j s", p=128))
                ct_t = cstream.tile([128, jn * SLOC], BF16, name=f"ct{g}", tag="ct")
                nc.sync.dma_start(
                    out=ct_t[:].rearrange("p (j s) -> p j s", j=jn),
                    in_=lnct_d[j0 * 128:(j0 + jn) * 128, :]
                        .rearrange("(j p) s -> p j s", p=128))
                for b in range(B):
                    nc.sync.dma_start(
                        out=xsl(b, j0, jn).rearrange("p (j e) -> p j e", j=jn),
                        in_=x_d[b, j0 * 128:(j0 + jn) * 128, :]
                            .rearrange("(j p) e -> p j e", p=128))
                if g == 0:
                    nc.sync.dma_start(
                        out=win_sb[:].rearrange("p (t e) -> p t e", t=4),
                        in_=win_d.rearrange("(t p) e -> p t e", p=128))
                    nc.sync.dma_start(
                        out=woutT_sb[:].rearrange("p (t i) -> p t i", t=4),
                        in_=woutT_d.rearrange("(t p) i -> p t i", p=128))
                # numer: per-j-tile sub (DVE) + exp (ACT) so each stepA
                # matmul gates on its own tile, not the whole group
                for jj in range(jn):
                    sb_t = sstream.tile([128, SLOC], BF16,
                                        name=f"sb{g}_{jj}", tag="sb")
                    nc.vector.tensor_sub(sb_t[:],
                                         ct_t[:, jj * SLOC:(jj + 1) * SLOC],
                                         dt_t[:, jj * SLOC:(jj + 1) * SLOC])
                    nc.scalar.activation(nsl(j0 + jj), sb_t[:],
                                         mybir.ActivationFunctionType.Exp)
                # stepA for this group's j-tiles
                if g < LASTG:
                    for j in range(j0, j0 + jn):
                        stepa_j(j, 0)
                        stepa_j(j, 1)
                else:
                    for j in range(j0, j0 + jn):
                        stepa_j(j, 0)
                    tt_copies(0, "dve")
                    nc.vector.reciprocal(rdT[:], pdT[:])
                    for j in range(j0, j0 + jn):
                        stepa_j(j, 1)
                if g == 0:
                    # WcT[e,i] = sum_a W_in[a,e] * W_outT[a,i] — after the
                    # first stepA group so PE starts on stream data ASAP
                    for ec in range(4):
                        pw = ptr.tile([128, D], F32, name=f"pw{ec}", tag="tr")
                        for at in range(4):
                            nc.tensor.matmul(
                                pw[:],
                                lhsT=win_sb[:, at * D + ec * 128:
                                            at * D + (ec + 1) * 128],
                                rhs=woutT_sb[:, at * D:(at + 1) * D],
                                start=(at == 0), stop=(at == 3))
                        nc.vector.tensor_copy(wcT[:, ec * D:(ec + 1) * D], pw[:])

            # residual inputs — late on the ACT queue, after the x stream,
            # so they don't steal DMA bandwidth from stepA's tail
            for b in range(B):
                nc.scalar.dma_start(
                    out=xres_sb[:, b * NSC * D:(b + 1) * NSC * D]
                        .rearrange("p (sc e) -> p sc e", sc=NSC),
                    in_=xres_d[b].rearrange("(sc p) e -> p sc e", p=128))
            if with_bias:
                for b in range(B):
                    for sc in range(NSC):
                        nc.vector.tensor_add(bsl(resb, b, sc),
                                             bsl(xres_sb, b, sc), bias_bc[:])
            res_src = resb if with_bias else xres_sb
            # tT copies for b=1: one on DVE, one on ACT (parallel)
            nc.vector.tensor_copy(
                tT[:, (1 * NE) * SLOC:(1 * NE + 2) * SLOC], pts[(1, 0)][:])
            nc.scalar.copy(
                tT[:, (1 * NE + 2) * SLOC:(1 * NE + 4) * SLOC], pts[(1, 1)][:])

            # ---- step B + fused epilogue + output DMA ----
            for b in range(B):
                for sc in range(NSC):
                    po = pa.tile([128, D], F32, name=f"po{b}_{sc}", tag="acc")
                    for et in range(NE):
                        nc.tensor.matmul(
                            po[:],
                            lhsT=tsl(tT, b, et)[:, sc * 128:(sc + 1) * 128],
                            rhs=wcT[:, et * D:(et + 1) * D],
                            start=(et == 0), stop=(et == 3))
                    nc.vector.scalar_tensor_tensor(
                        out=bsl(outbuf, b, sc),
                        in0=po[:],
                        scalar=rdT[:, sc:sc + 1],
                        in1=bsl(res_src, b, sc),
                        op0=MUL, op1=ADD)
                    dma_eng = nc.sync if b == 0 else nc.scalar
                    dma_eng.dma_start(
                        out=out_d[b, sc * 128:(sc + 1) * 128, :],
                        in_=bsl(outbuf, b, sc))

    nc.compile()
    return nc


def _get_nc(with_bias=True):
    key = ("nc", with_bias)
    if key not in _CACHE:
        _CACHE[key] = _build_nc(with_bias)
    return _CACHE[key]


def prep_in_maps(x, routes, distances, W_in, W_out, b_out):
    """Host-side sharding/marshalling: per-core input dicts."""
    import ml_dtypes
    import concourse.mybir as mybir

    bf16 = ml_dtypes.bfloat16
    f8 = mybir.dt.np(mybir.dt.float8e4 if STEPA_FP8 else mybir.dt.bfloat16)
    x = np.ascontiguousarray(np.asarray(x, dtype=np.float32))
    routes = np.asarray(routes, dtype=np.int32)
    distances = np.ascontiguousarray(np.asarray(distances, dtype=np.float32))
    W_in_b = np.ascontiguousarray(np.asarray(W_in, dtype=np.float32)).astype(bf16)
    W_outT_b = np.ascontiguousarray(
        np.asarray(W_out, dtype=np.float32).T).astype(bf16)
    b_out = np.ascontiguousarray(np.asarray(b_out, dtype=np.float32)).reshape(1, D)

    x_8 = x.astype(f8)

    # Count matrix C^T[j, s] = multiplicity of j in routes[s, :], shipped as
    # ln(C) so the device computes C*exp(-d) = exp(lnC - d); C=0 -> -1e4
    # underflows exp to +0. Depends only on the int32 index tensor.
    flat = routes.astype(np.int64).ravel() * S + np.repeat(np.arange(S, dtype=np.int64), K)
    countsT = np.bincount(flat, minlength=S * S).reshape(S, S)
    with np.errstate(divide="ignore"):
        lnctT = np.log(countsT.astype(np.float32))
    lnctT[countsT == 0] = -1e4
    lnctT = lnctT.astype(bf16)
    distT = np.ascontiguousarray(distances.T).astype(bf16)

    in_maps = []
    for c in range(N_CORES):
        sl = slice(c * SLOC, (c + 1) * SLOC)
        in_maps.append({
            "x": x_8,
            "distT": np.ascontiguousarray(distT[:, sl]),
            "lnct": np.ascontiguousarray(lnctT[:, sl]),
            "xres": np.ascontiguousarray(x[:, sl, :]),
            "w_in": W_in_b,
            "w_outT": W_outT_b,
            "b_out": b_out,
        })
    return in_maps


def kernel(x, routes, distances, W_in, W_out, b_out):
    global LAST_RESULTS
    from concourse import bass_utils

    in_maps = prep_in_maps(x, routes, distances, W_in, W_out, b_out)
    with_bias = bool(np.any(np.asarray(b_out)))
    nc = _get_nc(with_bias)
    _CACHE["last_nc"] = nc
    res = bass_utils.run_bass_kernel_spmd(nc, in_maps, core_ids=list(range(N_CORES)))
    LAST_RESULTS = res
    out = np.concatenate([res.results[c]["out"] for c in range(N_CORES)], axis=1)
    return out


if __name__ == "__main__":
    rng = np.random.default_rng(0)
    inputs = {
        "x": rng.standard_normal((B, S, D), dtype=np.float32),
        "routes": rng.integers(0, S, (S, K)).astype(np.int32),
        "distances": rng.random((S, S), dtype=np.float32),
        "W_in": (rng.standard_normal((D, D), dtype=np.float32) / np.sqrt(D)).astype(np.float32),
        "W_out": (rng.standard_normal((D, D), dtype=np.float32) / np.sqrt(D)).astype(np.float32),
        "b_out": np.zeros(D, dtype=np.float32),
    }
    out = kernel(**inputs)
    print("out", out.shape, out.dtype)



# revision 2
# speedup vs baseline: 1.3824x; 1.3824x over previous
"""Trainium2 Bass kernel for CantorMultiheadFusion.

Reference math:
    h      = x @ W_in^T                        # [B,S,D]
    d[s,k] = distances[s, routes[s,k]]
    w      = softmax(-d, axis=-1)              # [S,K]
    fused  = sum_k w[s,k] * h[:, routes[s,k]]  # [B,S,D]  (head reshape is a no-op)
    out    = fused @ W_out^T + b_out + x

Because the fusion weights are shared across the feature dim, the gather
commutes with both projections:
    out = (A @ x) @ (W_out @ W_in)^T + b_out + x
where A[s,j] = C[s,j] * exp(-distances[s,j]) / denom(s),
      C[s,j] = #{k : routes[s,k] == j}   (integer multiplicity),
      denom(s) = sum_j C[s,j] * exp(-distances[s,j]).
Duplicated route entries share the same distance, so the count matrix C is
exact. On device numer = exp(-d) * C with C shipped as small exact ints in
fp8 (C=0 kills masked entries, so d needs no masking). All float math on
the actual inputs (exp, C-multiply, normalization, matmuls, residual) runs
on device; the host only does index marshalling (bincount of routes,
transposes/layout packs) and dtype casts.

Sharding: sequence-parallel over S across 8 cores (256 rows each). x is
replicated since the A@x contraction needs all S rows.

Perf notes (driven by the TimelineSim cost model):
  - stepA (t^T[e,s] += x^T @ numer) runs in fp8e4 with
    perf_mode=DoubleRow: one matmul contracts TWO 128-row j-tiles at 0.5
    cycles/col -> 4x fewer PE cycles than bf16.
  - Every HBM tensor is pre-packed on host into its exact SBUF image
    [128, cols] so each stream is ONE big descriptor-friendly DMA
    (contiguous runs >= 2KB: full 360GB/s, minimal HWDGE occupancy).
  - Streams: x fp8 (2.1MB), distances^T bf16 (1.05MB), counts fp8
    (0.5MB), residual bf16 (0.5MB), weights bf16 (1MB), out written bf16
    (0.5MB, upcast to f32 on host) -> ~5.8MB/core vs 9.4MB baseline.
  - Batch 0's x streams first so stepB/epilogue/output-DMA for b=0
    overlap with b=1's stepA input stream; only b=1's short tail remains.

Measured rel err ~4e-3 (gate 2e-2).
"""

import os
import sys

import numpy as np

for _p in ("/opt/trn_rl_repo",):
    if os.path.isdir(_p) and _p not in sys.path:
        sys.path.insert(0, _p)

# Some container snapshots lack antenv.axon_hooks (the axon NTFF profile
# hook); stub it so run_bass_kernel_spmd(trace=True) degrades gracefully.
def _ensure_axon_hooks_stub():
    import types
    try:
        import antenv.axon_hooks  # noqa: F401
    except ModuleNotFoundError:
        try:
            import antenv
        except ModuleNotFoundError:
            return
        _stub = types.ModuleType("antenv.axon_hooks")
        _stub.get_axon_ntff_profile_hook = lambda: None
        sys.modules["antenv.axon_hooks"] = _stub
        antenv.axon_hooks = _stub


_ensure_axon_hooks_stub()

B, S, D, K = 2, 2048, 512, 64
N_CORES = 8
SLOC = S // N_CORES          # 256 sequence rows per core
NJ = S // 128                # 16 contraction j-tiles
NE = D // 128                # 4 feature chunks
NSC = SLOC // 128            # 2 seq chunks per core
NJP = NJ // 2                # 8 DoubleRow j-pairs
NG = 4                       # numer/x pipeline groups (4 j-tiles each)
GT = NJ // NG                # j-tiles per group

_CACHE = {}
LAST_RESULTS = None


def _build_nc(with_bias=True):
    import concourse.bacc as bacc
    import concourse.mybir as mybir
    import concourse.tile as tile

    F32 = mybir.dt.float32
    BF16 = mybir.dt.bfloat16
    F8 = mybir.dt.float8e4
    MUL = mybir.AluOpType.mult
    ADD = mybir.AluOpType.add
    DR = mybir.MatmulPerfMode.DoubleRow
    EXP = mybir.ActivationFunctionType.Exp

    nc = bacc.Bacc("TRN2", target_bir_lowering=False, debug=False, num_devices=1)

    # All inputs are host-packed SBUF images [128, cols].
    x8_d = nc.dram_tensor("x8", [128, B * NJ * D], F8, kind="ExternalInput").ap()
    mT_d = nc.dram_tensor("mT", [128, NJ * SLOC], BF16, kind="ExternalInput").ap()
    c8_d = nc.dram_tensor("c8", [128, NJ * SLOC], F8, kind="ExternalInput").ap()
    xres_d = nc.dram_tensor("xres", [128, B * NSC * D], BF16, kind="ExternalInput").ap()
    win_d = nc.dram_tensor("w_in", [128, NE * D], BF16, kind="ExternalInput").ap()
    woutT_d = nc.dram_tensor("w_outT", [128, NE * D], BF16, kind="ExternalInput").ap()
    bout_d = nc.dram_tensor("b_out", [1, D], F32, kind="ExternalInput").ap()
    out_d = nc.dram_tensor("out", [B, SLOC, D], BF16, kind="ExternalOutput").ap()

    with tile.TileContext(nc) as tc:
        with (
            tc.tile_pool(name="big", bufs=1) as big,
            tc.tile_pool(name="pa", bufs=4, space="PSUM") as pa,
            tc.tile_pool(name="pden", bufs=1, space="PSUM") as pdenp,
            tc.tile_pool(name="ptr", bufs=3, space="PSUM") as ptr,
        ):
            # ---- persistent SBUF ----
            # x8 col = b*8192 + jp*1024 + ec*256 + two*128 + e  (j = 2*jp+two)
            xbuf = big.tile([128, B * NJ * D], F8)
            mbuf = big.tile([128, NJ * SLOC], BF16)     # col = j*256 + s
            cbuf = big.tile([128, NJ * SLOC], F8)
            ebuf = big.tile([128, NJ * SLOC], BF16)     # exp(-m)
            numer = big.tile([128, NJ * SLOC], F8)      # exp(-m)*C
            win_sb = big.tile([128, NE * D], BF16)      # col = a_tile*512 + e
            woutT_sb = big.tile([128, NE * D], BF16)    # col = a_tile*512 + i
            wcT = big.tile([128, NE * D], BF16)         # col = e_tile*512 + i
            xres_sb = big.tile([128, B * NSC * D], BF16)
            tT = big.tile([128, B * NE * SLOC], BF16)   # col = (b*4+ec)*256 + s
            outbuf = big.tile([128, B * NSC * D], BF16)
            rdT = big.tile([128, NSC], F32)
            onescol = big.tile([128, 1], F8)
            bias_sb = big.tile([1, D], F32)
            bias_bc = big.tile([128, D], F32)
            ones_r = big.tile([1, 128], F32)
            resb = big.tile([128, B * NSC * D], BF16)

            nc.vector.memset(onescol[:], 1.0)

            # stepA PSUM: bank (b,p) holds ec=2p (cols :256) and ec=2p+1
            # (cols 256:). Exactly one start=True per bank.
            pdT = pdenp.tile([128, NSC], F32)
            pts = {(b, p): pa.tile([128, 2 * SLOC], F32,
                                   name=f"pts{b}_{p}", tag="acc")
                   for b in range(B) for p in range(NE // 2)}

            def pta(b, ec):
                return pts[(b, ec // 2)][:, (ec % 2) * SLOC:(ec % 2 + 1) * SLOC]

            def bsl(buf, b, sc):
                o = (b * NSC + sc) * D
                return buf[:, o:o + D]

            # ---- DMA stream (SP queue; device serves in this order) ----
            if with_bias:
                nc.sync.dma_start(out=bias_sb[:1, :], in_=bout_d[:, :])
                nc.vector.memset(ones_r[:], 1.0)
                pb = ptr.tile([128, D], F32, name="pb", tag="tr")
                nc.tensor.matmul(pb[:], lhsT=ones_r[:1, :], rhs=bias_sb[:1, :],
                                 start=True, stop=True)
                nc.vector.tensor_copy(bias_bc[:], pb[:])

            def mc_dma(g0, gn):
                nc.sync.dma_start(
                    out=mbuf[:, g0 * GT * SLOC:(g0 + gn) * GT * SLOC],
                    in_=mT_d[:, g0 * GT * SLOC:(g0 + gn) * GT * SLOC])
                nc.sync.dma_start(
                    out=cbuf[:, g0 * GT * SLOC:(g0 + gn) * GT * SLOC],
                    in_=c8_d[:, g0 * GT * SLOC:(g0 + gn) * GT * SLOC])

            def x_dma(b, g):
                o = b * (NJ * D) + g * GT * D
                nc.sync.dma_start(out=xbuf[:, o:o + GT * D],
                                  in_=x8_d[:, o:o + GT * D])

            mc_dma(0, 2)
            x_dma(0, 0)
            x_dma(0, 1)
            nc.sync.dma_start(out=win_sb[:], in_=win_d[:, :])
            nc.sync.dma_start(out=woutT_sb[:], in_=woutT_d[:, :])
            mc_dma(2, 2)
            x_dma(0, 2)
            x_dma(0, 3)
            for g in range(NG):
                x_dma(1, g)
            nc.sync.dma_start(out=xres_sb[:], in_=xres_d[:, :])

            # ---- numer prep: exp on ACT, C-multiply on DVE (per group) ----
            for g in range(NG):
                sl = slice(g * GT * SLOC, (g + 1) * GT * SLOC)
                nc.scalar.activation(ebuf[:, sl], mbuf[:, sl], EXP, scale=-1.0)
            for g in range(NG):
                sl = slice(g * GT * SLOC, (g + 1) * GT * SLOC)
                nc.vector.tensor_mul(numer[:, sl], ebuf[:, sl], cbuf[:, sl])

            # ---- PE ----
            def stepa_group(b, g):
                for jp in range(g * GT // 2, (g + 1) * GT // 2):
                    rhs = numer[:, jp * 2 * SLOC:(jp + 1) * 2 * SLOC] \
                        .rearrange("p (two s) -> p two s", two=2)
                    for ec in range(NE):
                        o = b * (NJ * D) + jp * 1024 + ec * 256
                        lhsT = xbuf[:, o:o + 256] \
                            .rearrange("p (two e) -> p two e", two=2)
                        nc.tensor.matmul(
                            pta(b, ec), lhsT=lhsT, rhs=rhs,
                            start=(jp == 0 and ec % 2 == 0),
                            stop=(jp == NJP - 1 and ec % 2 == 1),
                            perf_mode=DR, skip_group_check=True)

            def denom_group(g):
                for j in range(g * GT, (g + 1) * GT):
                    for sc in range(NSC):
                        nc.tensor.matmul(
                            pdT[:, sc:sc + 1],
                            lhsT=numer[:, j * SLOC + sc * 128:
                                       j * SLOC + (sc + 1) * 128],
                            rhs=onescol[:, :1],
                            start=(j == 0 and sc == 0),
                            stop=(j == NJ - 1 and sc == NSC - 1),
                            skip_group_check=True)

            def wct():
                # WcT[e,i] = sum_a W_in[a,e] * W_outT[a,i]
                pws = []
                for ec in range(NE):
                    pw = ptr.tile([128, D], F32, name=f"pw{ec}", tag="tr")
                    pws.append(pw)
                    for at in range(NE):
                        nc.tensor.matmul(
                            pw[:],
                            lhsT=win_sb[:, at * D + ec * 128:
                                        at * D + (ec + 1) * 128],
                            rhs=woutT_sb[:, at * D:(at + 1) * D],
                            start=(at == 0), stop=(at == NE - 1))
                return pws

            denom_group(0)
            stepa_group(0, 0)
            denom_group(1)
            stepa_group(0, 1)
            pws = wct()
            denom_group(2)
            stepa_group(0, 2)
            denom_group(3)
            stepa_group(0, 3)
            for g in range(NG):
                stepa_group(1, g)

            # wcT PSUM->SBUF copies: split DVE/ACT
            nc.vector.tensor_copy(wcT[:, 0 * D:1 * D], pws[0][:])
            nc.scalar.copy(wcT[:, 1 * D:2 * D], pws[1][:])
            nc.vector.tensor_copy(wcT[:, 2 * D:3 * D], pws[2][:])
            nc.scalar.copy(wcT[:, 3 * D:4 * D], pws[3][:])

            # tT copies (PSUM f32 -> SBUF bf16), b0 first
            nc.vector.tensor_copy(tT[:, 0:512], pts[(0, 0)][:])
            nc.scalar.copy(tT[:, 512:1024], pts[(0, 1)][:])
            nc.vector.reciprocal(rdT[:], pdT[:])
            nc.vector.tensor_copy(tT[:, 1024:1536], pts[(1, 0)][:])
            nc.scalar.copy(tT[:, 1536:2048], pts[(1, 1)][:])

            if with_bias:
                for b in range(B):
                    for sc in range(NSC):
                        nc.vector.tensor_add(bsl(resb, b, sc),
                                             bsl(xres_sb, b, sc), bias_bc[:])
            res_src = resb if with_bias else xres_sb

            # ---- step B + fused epilogue + output DMA ----
            for b in range(B):
                for sc in range(NSC):
                    po = pa.tile([128, D], F32, name=f"po{b}_{sc}", tag="acc")
                    for et in range(NE):
                        nc.tensor.matmul(
                            po[:],
                            lhsT=tT[:, (b * NE + et) * SLOC + sc * 128:
                                    (b * NE + et) * SLOC + (sc + 1) * 128],
                            rhs=wcT[:, et * D:(et + 1) * D],
                            start=(et == 0), stop=(et == NE - 1))
                    nc.vector.scalar_tensor_tensor(
                        out=bsl(outbuf, b, sc),
                        in0=po[:],
                        scalar=rdT[:, sc:sc + 1],
                        in1=bsl(res_src, b, sc),
                        op0=MUL, op1=ADD)
                    nc.sync.dma_start(
                        out=out_d[b, sc * 128:(sc + 1) * 128, :],
                        in_=bsl(outbuf, b, sc))

    nc.compile()
    return nc


def _get_nc(with_bias=True):
    key = ("nc", with_bias)
    if key not in _CACHE:
        _CACHE[key] = _build_nc(with_bias)
    return _CACHE[key]


def prep_in_maps(x, routes, distances, W_in, W_out, b_out):
    """Host-side sharding/marshalling: per-core input dicts.

    Pure index marshalling + dtype casts only: bincount over the int32
    routes tensor, transposes/reshapes, and bf16/fp8 casts. No float
    arithmetic on any input values.
    """
    import ml_dtypes
    import concourse.mybir as mybir

    bf16 = ml_dtypes.bfloat16
    f8 = mybir.dt.np(mybir.dt.float8e4)
    x = np.ascontiguousarray(np.asarray(x, dtype=np.float32))
    routes = np.asarray(routes, dtype=np.int32)
    distances = np.ascontiguousarray(np.asarray(distances, dtype=np.float32))
    b_out = np.ascontiguousarray(np.asarray(b_out, dtype=np.float32)).reshape(1, D)

    # x8 SBUF image: [p, b*8192 + jp*1024 + ec*256 + two*128 + e],
    # x[b, (2*jp+two)*128 + p, ec*128+e]  (DoubleRow-interleaved pairs)
    x8 = np.asarray(x, dtype=f8).reshape(B, NJP, 2, 128, NE, 128)
    x8 = np.ascontiguousarray(x8.transpose(3, 0, 1, 4, 2, 5)).reshape(128, B * NJ * D)

    # W_in [p, t*512+e] = W_in[t*128+p, e]; W_outT [p, t*512+i] = W_out[i, t*128+p]
    W_in_b = np.asarray(W_in, dtype=np.float32).astype(bf16)
    W_in_b = np.ascontiguousarray(
        W_in_b.reshape(NE, 128, D).transpose(1, 0, 2)).reshape(128, NE * D)
    W_outT_b = np.asarray(W_out, dtype=np.float32).T.astype(bf16)
    W_outT_b = np.ascontiguousarray(
        W_outT_b.reshape(NE, 128, D).transpose(1, 0, 2)).reshape(128, NE * D)

    # Count matrix C^T[j, s] = multiplicity of j in routes[s, :]; exact
    # small ints, shipped as fp8 (asserted <= 16 so the cast is exact).
    flat = routes.astype(np.int64).ravel() * S + \
        np.repeat(np.arange(S, dtype=np.int64), K)
    countsT = np.bincount(flat, minlength=S * S).reshape(S, S)
    assert countsT.max() <= 16, "fp8e4m3 exact-int range exceeded"
    # [j_full, s] -> [j_tile, p, s] -> [p, j_tile, s]
    cT = countsT.astype(f8).reshape(NJ, 128, S).transpose(1, 0, 2)
    dT = distances.T.astype(bf16).reshape(NJ, 128, S).transpose(1, 0, 2)

    in_maps = []
    for c in range(N_CORES):
        sl = slice(c * SLOC, (c + 1) * SLOC)
        xres = x[:, sl, :].reshape(B, NSC, 128, D).transpose(2, 0, 1, 3)
        in_maps.append({
            "x8": x8,
            "mT": np.ascontiguousarray(dT[:, :, sl]).reshape(128, NJ * SLOC),
            "c8": np.ascontiguousarray(cT[:, :, sl]).reshape(128, NJ * SLOC),
            "xres": np.ascontiguousarray(xres.astype(bf16)).reshape(
                128, B * NSC * D),
            "w_in": W_in_b,
            "w_outT": W_outT_b,
            "b_out": b_out,
        })
    return in_maps


def kernel(x, routes, distances, W_in, W_out, b_out):
    global LAST_RESULTS
    from concourse import bass_utils

    in_maps = prep_in_maps(x, routes, distances, W_in, W_out, b_out)
    with_bias = bool(np.any(np.asarray(b_out)))
    nc = _get_nc(with_bias)
    _CACHE["last_nc"] = nc
    res = bass_utils.run_bass_kernel_spmd(nc, in_maps, core_ids=list(range(N_CORES)))
    LAST_RESULTS = res
    out = np.concatenate(
        [np.asarray(res.results[c]["out"]) for c in range(N_CORES)],
        axis=1).astype(np.float32)
    return out


if __name__ == "__main__":
    rng = np.random.default_rng(0)
    inputs = {
        "x": rng.standard_normal((B, S, D), dtype=np.float32),
        "routes": rng.integers(0, S, (S, K)).astype(np.int32),
        "distances": rng.random((S, S), dtype=np.float32),
        "W_in": (rng.standard_normal((D, D), dtype=np.float32) / np.sqrt(D)).astype(np.float32),
        "W_out": (rng.standard_normal((D, D), dtype=np.float32) / np.sqrt(D)).astype(np.float32),
        "b_out": np.zeros(D, dtype=np.float32),
    }
    out = kernel(**inputs)
    print("out", out.shape, out.dtype)
